# revision 23
# baseline (speedup 1.0000x reference)
"""Trainium2 Bass kernel for nn_EICLayer2 (gnn_message_passing).

Computation (per batch element b):
  rows 0-2: for each (row, col2): y[b,row,col2,:] = sigmoid(z - 0.5*max_g(z))
            where z = chunk[b,row,col2,:] @ W256[row*4+col2].T
            and chunk[...,l1c*64+k] = x[b,row,l1c,col2*64+k]
  row 3:    same with only l1c in {0,1,2} (192 input features), W192.

Strategy: pure data-parallel over batch across 8 cores (2048 each).
Per core, per 128-batch tile:
  DMA x (fp32, first 3840 of 4096 features; the (row3,l1c3) chunk is unused)
  -> cast+swizzle to fp16 so each (row,col2) 256-feature chunk is contiguous
  -> PE transpose (fp16) to put features on partitions -> one batched DVE
  copy PSUM->SBUF per 4 chunks -> 2 accumulating fp16 matmuls per chunk
  (stationary = x^T chunk, moving = pre-transposed weights) -> batched DVE
  max-reduce + ACT sigmoid epilogue -> DMA out (fp16, host-upcast).

Weights are tiny (<4MB); they are pre-transposed/padded to fp16 on host and
replicated to all cores.
"""

import numpy as np

B = 16384
N_CORES = 8
B_CORE = B // N_CORES  # 2048
P = 128

# knobs for experimentation
TRACE = False
STITCH = False
LAST_RESULTS = None  # BassKernelResults of last run


def _build_bass(b_core=B_CORE, variant="full"):
    import concourse.mybir as mybir
    import concourse.tile as tile
    from concourse import bacc
    from concourse.bass import ts
    from concourse.masks import make_identity

    fp32 = mybir.dt.float32
    fp16 = mybir.dt.float16

    n_tiles = b_core // P

    nc = bacc.Bacc("TRN2", target_bir_lowering=False, debug=False)
    x_d = nc.dram_tensor("x", [b_core, 4, 4, 256], fp32, kind="ExternalInput")
    # host pre-swizzled: wt_d[p, rc, j, g] = W^T[rc][j*128+p, g]
    wt_d = nc.dram_tensor("wt", [P, 16, 2, 256], fp16, kind="ExternalInput")
    y_d = nc.dram_tensor("y", [b_core, 4, 4, 256], fp16, kind="ExternalOutput")

    x_tiled = x_d.rearrange("(t p) r c f -> t p (r c f)", p=P)  # [T, 128, 4096]
    y_tiled = y_d.rearrange("(t p) r c f -> t p (r c f)", p=P)
    wt_view = wt_d[:]

    with tile.TileContext(nc) as tc:
        with (
            tc.tile_pool(name="singles", bufs=1) as singles,
            tc.tile_pool(name="xin", bufs=3) as xin_pool,
            tc.tile_pool(name="x16", bufs=2) as x16_pool,
            tc.tile_pool(name="xt", bufs=4) as xt_pool,
            tc.tile_pool(name="yout", bufs=2) as y_pool,
            tc.tile_pool(name="mx", bufs=3) as mx_pool,
            tc.tile_pool(name="pt", bufs=2, space="PSUM") as pt_pool,
            tc.tile_pool(name="py", bufs=3, space="PSUM") as py_pool,
        ):
            ident = singles.tile([P, P], fp16)
            make_identity(nc, ident)
            wt_sb = singles.tile([P, 16, 2, 256], fp16)
            nc.sync.dma_start(out=wt_sb, in_=wt_view)

            for t in range(n_tiles):
                # skip the unused (row3, l1c3) chunk: contiguous 3840 prefix
                x32 = xin_pool.tile([P, 3840], fp32)
                nc.sync.dma_start(out=x32, in_=x_tiled[t][:, 0:3840])

                # x16[p, r, c, l*64+k] = x32[p, r*1024 + l*256 + c*64 + k]
                x16 = x16_pool.tile([P, 4, 4, 256], fp16)
                x32v = x32[:, 0:3072].rearrange(
                    "p (r l c k) -> p r c l k", r=3, l=4, c=4
                )
                x32v3 = x32[:, 3072:3840].rearrange(
                    "p (l c k) -> p c l k", l=3, c=4
                )
                for r in range(4):
                    nl = 4 if r < 3 else 3
                    src = x32v[:, r] if r < 3 else x32v3
                    dst = x16[:, r, :, 0 : nl * 64].rearrange(
                        "p c (l k) -> p c l k", l=nl
                    )
                    if r == 2:
                        nc.vector.tensor_copy(out=dst[:, 0:2], in_=src[:, 0:2])
                        nc.gpsimd.tensor_copy(out=dst[:, 2:4], in_=src[:, 2:4])
                    else:
                        nc.gpsimd.tensor_copy(out=dst, in_=src)
                # zero the (row3, l1c3) feature lanes so transposed garbage
                # can't poison the zero-weight matmul rows
                nc.gpsimd.memset(x16[:, 3, :, 192:256], 0.0)

                y_sb = y_pool.tile([P, 4096], fp16)
                for grp in range(4):
                    mx = mx_pool.tile([P, 4], fp32, tag="mx")
                    nb = mx_pool.tile([P, 4], fp32, tag="nb")

                    # 8 PE transposes into one PSUM bank, one batched copyback
                    pt = pt_pool.tile([P, 4, 256], fp16)
                    for i in range(4):
                        rc = grp * 4 + i
                        r, c = rc // 4, rc % 4
                        for j in range(2):
                            nc.tensor.transpose(
                                pt[:, i, ts(j, P)], x16[:, r, c, ts(j, P)], ident
                            )
                    xt = xt_pool.tile([P, 4, 2, P], fp16)
                    nc.vector.tensor_copy(
                        out=xt.rearrange("p i j b -> p (i j b)"),
                        in_=pt.rearrange("p i f -> p (i f)"),
                    )

                    py = py_pool.tile([P, 4, 256], fp32)
                    for i in range(4):
                        rc = grp * 4 + i
                        nc.tensor.matmul(
                            py[:, i, :], xt[:, i, 0, :], wt_sb[:, rc, 0, :],
                            start=True, stop=False,
                        )
                        nc.tensor.matmul(
                            py[:, i, :], xt[:, i, 1, :], wt_sb[:, rc, 1, :],
                            start=False, stop=True,
                        )
                    nc.vector.reduce_max(mx, py, axis=mybir.AxisListType.X)
                    nc.scalar.mul(nb, mx, -0.5)
                    for i in range(4):
                        rc = grp * 4 + i
                        nc.scalar.activation(
                            out=y_sb[:, ts(rc, 256)],
                            in_=py[:, i, :],
                            func=mybir.ActivationFunctionType.Sigmoid,
                            bias=nb[:, i : i + 1],
                            scale=1.0,
                        )
                nc.scalar.dma_start(out=y_tiled[t], in_=y_sb)
    nc.compile()
    return nc


def _prep_weights(W256, W192):
    wt = np.zeros((16, 256, 256), np.float16)
    w256 = np.asarray(W256, np.float32).reshape(3, 4, 256, 256)  # [r, c, g, f]
    for r in range(3):
        for c in range(4):
            wt[r * 4 + c] = w256[r, c].T.astype(np.float16)  # [f, g]
    w192 = np.asarray(W192, np.float32)  # [c, g, f]
    for c in range(4):
        wt[12 + c, 0:192, :] = w192[c].T.astype(np.float16)
    # swizzle to DMA-friendly layout: [p, rc, j, g] = wt[rc, j*128+p, g]
    return np.ascontiguousarray(wt.reshape(16, 2, P, 256).transpose(2, 0, 1, 3))


def kernel(x, W256, W192):
    global LAST_RESULTS
    from concourse.bass_utils import run_bass_kernel_spmd

    x = np.ascontiguousarray(np.asarray(x, np.float32))
    wt = _prep_weights(W256, W192)

    nc = _build_bass()
    in_maps = [
        {"x": x[i * B_CORE : (i + 1) * B_CORE], "wt": wt} for i in range(N_CORES)
    ]
    res = run_bass_kernel_spmd(
        nc,
        in_maps,
        core_ids=list(range(N_CORES)),
        trace=TRACE,
        stitch_traces=STITCH,
    )
    LAST_RESULTS = res
    out = np.concatenate([r["y"] for r in res.results], axis=0)
    # y is stored fp16 on-chip to halve output DMA traffic; upcast on host
    return out.astype(np.float32)
